# revision 1
# baseline (speedup 1.0000x reference)
"""CapsNet dynamic-routing kernel for 8 Trainium2 NeuronCores.

Strategy: shard n_routes (4096) across 8 cores (512 routes/core).
u_hat is computed once per core via PE matmuls (block-diagonal x as the
stationary operand, host-pre-transposed W as the streaming operand) and
kept resident in SBUF as [p=(rb,b), (g,c,o)] fp32 (128 KB/partition).
Routing iterations use:
  - PE delta-matrix matmuls for all cross-partition sums (sum over rb,
    batch-mean, partition replication)
  - DVE tensor_tensor + tensor_reduce scans over u_hat for the
    c_ij-weighted route sums and the agreement terms
  - tiny AllReduce (16x544 f32) per iteration for the global route-sum
    and softmax denominator (unnormalized-softmax trick: exp(b_ij)
    without max subtraction; b_ij stays O(10) for this problem family)
"""

import numpy as np

B, R, I, C, O = 16, 4096, 16, 32, 16
NCORES = 8
RL = R // NCORES      # 512 routes per core
G = RL // 8           # 64 groups of 8 routes
CO = C * O            # 512
CHG = 4               # groups per DVE chunk
NCH = G // CHG        # 16 chunks

_cache = {}


def _patch_tile_drain():
    import concourse.tile as tile_mod
    from concourse.vector_clock import ScopedClock, VectorClock

    if getattr(tile_mod.TileContext, "_drain_patched", False):
        return

    def _split_drain_and_barrier(self, tick_clock, wait_clock):
        ticks = list(tick_clock.global_clock)
        for i in [j for j, t in enumerate(ticks) if t > 0]:
            vec = [ticks[j] if j == i else 0 for j in range(len(ticks))]
            d = self.nc.sync.drain()
            wait_clock.add_sem_waits(d.ins, ScopedClock({None: VectorClock(vec)}))
        self.nc.all_engine_barrier()
        popped = self.nc._tile_sem_poison_stack.pop()
        assert popped is self._sem_poison
        self.nc.clear_and_free_semaphores(list(self.sems.allocated().values()))
        self.nc.all_engine_barrier()

    tile_mod.TileContext._drain_and_barrier = _split_drain_and_barrier
    tile_mod.TileContext._drain_patched = True


def _split_waits(nc, limit=1):
    """This container's walrus rejects >1 sync-wait per instruction; move
    excess waits onto same-engine NoOps inserted just before the owner."""
    import concourse.mybir as mybir

    blocks = nc.main_func.blocks
    for bb in blocks:
        insts = bb.instructions  # live list view
        k = 0
        while k < len(insts):
            inst = insts[k]
            si = inst.sync_info
            if si is not None and si.on_wait and len(si.on_wait) > limit:
                w = list(si.on_wait)
                si.on_wait = w[:limit]
                excess = w[limit:]
                insert_at = k
                for cs in range(0, len(excess), limit):
                    chunk = excess[cs:cs + limit]
                    nop = nc.engines[inst.engine].nop()
                    ni = nop.ins
                    for bb2 in blocks:
                        l2 = bb2.instructions
                        hit = next(
                            (i for i in range(len(l2) - 1, -1, -1)
                             if l2[i].name == ni.name), None)
                        if hit is not None:
                            l2.pop(hit)
                            break
                    ni.sync_info = mybir.SyncInfo(on_wait=chunk, on_update=[])
                    insts.insert(insert_at, ni)
                    insert_at += 1
                    k += 1
            k += 1


def _build_nc():
    import concourse.bass as bass
    import concourse.mybir as mybir
    from concourse.tile import TileContext

    _patch_tile_drain()
    F32 = mybir.dt.float32
    AF = mybir.ActivationFunctionType
    ALU = mybir.AluOpType
    CORES = list(range(NCORES))

    nc = bass.Bass(target_bir_lowering=False)
    wt_d = nc.declare_dram_parameter("wt", [G, 128, CO], F32, isOutput=False)
    xb_d = nc.declare_dram_parameter("xb", [G, 128, 128], F32, isOutput=False)
    db_d = nc.declare_dram_parameter("delta_b", [128, 16], F32, isOutput=False)
    dbs_d = nc.declare_dram_parameter("delta_bs", [128, 16], F32, isOutput=False)
    ob_d = nc.declare_dram_parameter("ones_bd", [128, 128], F32, isOutput=False)
    o16_d = nc.declare_dram_parameter("ones_16", [128, 16], F32, isOutput=False)
    dr_d = nc.declare_dram_parameter("delta_rep", [16, 128], F32, isOutput=False)
    out_d = nc.declare_dram_parameter("out", [16, CO], F32, isOutput=True)
    cc_in = [nc.dram_tensor(f"cc_in{k}", [16, 544], F32) for k in range(3)]
    cc_out = [
        nc.dram_tensor(f"cc_out{k}", [16, 544], F32, addr_space="Shared")
        for k in range(3)
    ]

    with TileContext(nc) as tc:
        with (
            tc.tile_pool(name="big", bufs=1) as big,
            tc.tile_pool(name="stage", bufs=3) as stage,
            tc.tile_pool(name="work", bufs=2) as work,
            tc.tile_pool(name="small", bufs=1) as small,
            tc.tile_pool(name="psum", bufs=1, space="PSUM") as psum,
            tc.tile_pool(name="psum_u", bufs=2, space="PSUM") as psum_u_pool,
        ):
            # constants
            db = small.tile([128, 16], F32, tag="db")
            dbs = small.tile([128, 16], F32, tag="dbs")
            ob = small.tile([128, 128], F32, tag="ob")
            o16 = small.tile([128, 16], F32, tag="o16")
            dr = small.tile([16, 128], F32, tag="dr")
            nc.sync.dma_start(out=db[:, :], in_=db_d[:, :])
            nc.sync.dma_start(out=dbs[:, :], in_=dbs_d[:, :])
            nc.sync.dma_start(out=ob[:, :], in_=ob_d[:, :])
            nc.sync.dma_start(out=o16[:, :], in_=o16_d[:, :])
            nc.sync.dma_start(out=dr[:, :], in_=dr_d[:, :])

            u_sb = big.tile([128, G, C, O], F32, tag="u")
            b_rep = small.tile([128, C, G], F32, tag="b_rep")
            e_rep = small.tile([128, C, G], F32, tag="e_rep")
            q = small.tile([128, C, G], F32, tag="q")
            nc.vector.memset(b_rep[:, :, :], 0.0)

            # ---- production + iteration-0 route-sum ----
            ps_s = psum.tile([16, CO], F32, tag="ps_s")
            for g in range(G):
                wt_t = stage.tile([128, CO], F32, tag="wt")
                xb_t = stage.tile([128, 128], F32, tag="xb")
                nc.sync.dma_start(out=wt_t[:, :], in_=wt_d[g, :, :])
                nc.sync.dma_start(out=xb_t[:, :], in_=xb_d[g, :, :])
                pu = psum_u_pool.tile([128, C * O], F32, tag="pu")
                nc.tensor.matmul(
                    pu[:, :], xb_t[:, :], wt_t[:, :],
                    start=True, stop=True,
                )
                puv = pu[:, :].rearrange("p (c o) -> p c o", c=C)
                if g % 2 == 0:
                    nc.vector.tensor_copy(u_sb[:, g, :, :], puv)
                else:
                    nc.scalar.copy(u_sb[:, g, :, :], puv)
                # s0 += (1/R) * sum_rb u  (delta_bs folds the 1/R)
                nc.tensor.matmul(
                    ps_s[:, :], dbs[:, :],
                    u_sb[:, g, :, :].rearrange("p c o -> p (c o)"),
                    start=(g == 0), stop=(g == G - 1),
                )

            st = small.tile([16, 544], F32, tag="st")
            st2 = small.tile([16, 544], F32, tag="st2")
            nc.vector.tensor_copy(st[:, :512], ps_s[:, :])
            nc.vector.memset(st[:, 512:], 0.0)
            nc.sync.dma_start(out=cc_in[0][:, :], in_=st[:, :])
            nc.gpsimd.collective_compute(
                "AllReduce", ALU.add, replica_groups=[CORES],
                ins=[cc_in[0][:, :]], outs=[cc_out[0][:, :]],
            )
            nc.sync.dma_start(out=st2[:, :], in_=cc_out[0][:, :])

            s_t = small.tile([16, CO], F32, tag="s")
            sq = small.tile([16, CO], F32, tag="sq")
            rt = small.tile([16, CO], F32, tag="rt")
            num = small.tile([16, CO], F32, tag="num")
            den = small.tile([16, CO], F32, tag="den")
            v_t = small.tile([16, CO], F32, tag="v")
            rdn = small.tile([16, C], F32, tag="rdn")
            v_rep = small.tile([128, C, O], F32, tag="v_rep")

            def squash(k):
                # v = sq*s / ((1+sq)*sqrt(sq)) computed like the reference
                if k == 0:
                    nc.vector.tensor_copy(s_t[:, :], st2[:, :512])
                else:
                    nc.vector.reciprocal(rdn[:, :], st2[:, 512:544])
                    nc.vector.tensor_tensor(
                        s_t[:, :].rearrange("p (c o) -> p c o", c=C),
                        st2[:, :512].rearrange("p (c o) -> p c o", c=C),
                        rdn[:, :].unsqueeze(2).broadcast_to([16, C, O]),
                        ALU.mult,
                    )
                nc.vector.tensor_tensor(sq[:, :], s_t[:, :], s_t[:, :], ALU.mult)
                nc.scalar.activation(rt[:, :], sq[:, :], AF.Sqrt)
                nc.vector.tensor_tensor(num[:, :], sq[:, :], s_t[:, :], ALU.mult)
                nc.vector.tensor_scalar_add(den[:, :], sq[:, :], 1.0)
                nc.vector.tensor_tensor(den[:, :], den[:, :], rt[:, :], ALU.mult)
                nc.vector.reciprocal(den[:, :], den[:, :])
                nc.vector.tensor_tensor(v_t[:, :], num[:, :], den[:, :], ALU.mult)

            for it in (1, 2):
                squash(it - 1)
                # v_rep[(rb,b), (c,o)] = v[b, (c,o)]
                ps_vr = psum.tile([128, C * O], F32, tag="ps_vr")
                nc.tensor.matmul(
                    ps_vr[:, :], dr[:, :], v_t[:, :],
                    start=True, stop=True,
                )
                nc.vector.tensor_copy(
                    v_rep[:, :, :],
                    ps_vr[:, :].rearrange("p (c o) -> p c o", c=C),
                )

                # ---- a-pass: q[p,(c,g)] = sum_o u * v_rep ----
                for ch in range(NCH):
                    prod = work.tile([128, CHG, C, O], F32, tag="prod")
                    gs = ch * CHG
                    nc.vector.tensor_tensor(
                        prod[:, :, :, :],
                        u_sb[:, gs:gs + CHG, :, :],
                        v_rep[:, :, :].unsqueeze(1).broadcast_to([128, CHG, C, O]),
                        ALU.mult,
                    )
                    nc.vector.tensor_reduce(
                        q[:, :, gs:gs + CHG].transpose([0, 2, 1]),
                        prod[:, :, :, :],
                        mybir.AxisListType.X,
                        ALU.add,
                    )
                # a_rep[(rb,b),(c,g)] = (1/B) sum_b' q[(rb,b'),(c,g)]
                qf = q[:, :, :].rearrange("p c g -> p (c g)")
                bf = b_rep[:, :, :].rearrange("p c g -> p (c g)")
                for j in range(4):
                    ps_ar = psum_u_pool.tile([128, 512], F32, tag="pu")
                    nc.tensor.matmul(
                        ps_ar[:, :],
                        ob[:, :], qf[:, j * 512:(j + 1) * 512],
                        start=True, stop=True,
                    )
                    nc.vector.tensor_tensor(
                        bf[:, j * 512:(j + 1) * 512],
                        bf[:, j * 512:(j + 1) * 512],
                        ps_ar[:, :], ALU.add,
                    )
                nc.scalar.activation(
                    e_rep[:, :, :].rearrange("p c g -> p (c g)"),
                    b_rep[:, :, :].rearrange("p c g -> p (c g)"),
                    AF.Exp,
                )
                # local softmax denominator: dn16[b,c] = sum_{local r} e
                dn = small.tile([128, C], F32, tag="dn")
                nc.vector.tensor_reduce(
                    dn[:, :], e_rep[:, :, :], mybir.AxisListType.X, ALU.add,
                )
                ps_dn = psum.tile([16, C], F32, tag="ps_dn")
                nc.tensor.matmul(
                    ps_dn[:, :], o16[:, :], dn[:, :], start=True, stop=True,
                )

                # ---- s-tilde pass: st[b,(c,o)] = sum_{g,rb} e * u ----
                ps_s2 = psum.tile([16, CO], F32, tag="ps_s")
                for ch in range(NCH):
                    prod2 = work.tile([128, CHG, C, O], F32, tag="prod")
                    gs = ch * CHG
                    nc.vector.tensor_tensor(
                        prod2[:, :, :, :],
                        u_sb[:, gs:gs + CHG, :, :],
                        e_rep[:, :, gs:gs + CHG].transpose([0, 2, 1])
                        .unsqueeze(3).broadcast_to([128, CHG, C, O]),
                        ALU.mult,
                    )
                    for gg in range(CHG):
                        gi = gs + gg
                        nc.tensor.matmul(
                            ps_s2[:, :], db[:, :],
                            prod2[:, gg, :, :].rearrange("p c o -> p (c o)"),
                            start=(gi == 0), stop=(gi == G - 1),
                        )
                nc.vector.tensor_copy(st[:, :512], ps_s2[:, :])
                nc.vector.tensor_copy(st[:, 512:544], ps_dn[:, :])
                nc.sync.dma_start(out=cc_in[it][:, :], in_=st[:, :])
                nc.gpsimd.collective_compute(
                    "AllReduce", ALU.add, replica_groups=[CORES],
                    ins=[cc_in[it][:, :]], outs=[cc_out[it][:, :]],
                )
                nc.sync.dma_start(out=st2[:, :], in_=cc_out[it][:, :])

            squash(2)
            nc.sync.dma_start(out=out_d[:, :], in_=v_t[:, :])

    _split_waits(nc)
    return nc


def _prep_inputs(x, W):
    x = np.ascontiguousarray(x, np.float32)
    W = np.ascontiguousarray(W, np.float32)
    # wt[core, g, (rb,i), (c,o)]
    Wv = W.reshape(NCORES, G, 8, C, O, I)
    wt = np.ascontiguousarray(
        Wv.transpose(0, 1, 2, 5, 3, 4).reshape(NCORES, G, 128, CO)
    )
    # xb[core, g, rb*16+i, rb*16+b] = x[b, r, i]
    xv = np.ascontiguousarray(x.transpose(1, 2, 0)).reshape(NCORES, G, 8, I, B)
    xb = np.zeros((NCORES, G, 128, 128), np.float32)
    for rb in range(8):
        xb[:, :, rb * 16:(rb + 1) * 16, rb * 16:(rb + 1) * 16] = xv[:, :, rb]
    db = np.tile(np.eye(16, dtype=np.float32), (8, 1))           # [128,16]
    dbs = db / np.float32(R)
    ob = np.kron(np.eye(8, dtype=np.float32),
                 np.full((16, 16), 1.0 / B, np.float32))          # [128,128]
    o16 = np.full((128, 16), 1.0 / 16.0, np.float32)
    dr = np.tile(np.eye(16, dtype=np.float32), (1, 8))            # [16,128]
    in_maps = []
    for c in range(NCORES):
        in_maps.append({
            "wt": wt[c], "xb": xb[c],
            "delta_b": db, "delta_bs": dbs, "ones_bd": ob,
            "ones_16": o16, "delta_rep": dr,
        })
    return in_maps


def kernel(x, W):
    from concourse.bass_utils import run_bass_kernel_spmd

    if "nc" not in _cache:
        _cache["nc"] = _build_nc()
    in_maps = _prep_inputs(x, W)
    res = run_bass_kernel_spmd(_cache["nc"], in_maps, list(range(NCORES)))
    v = res.results[0]["out"].reshape(B, C, O)[..., None]
    return np.ascontiguousarray(v, np.float32)



# revision 4
# speedup vs baseline: 1.9463x; 1.9463x over previous
"""CapsNet dynamic-routing kernel for 8 Trainium2 NeuronCores.

Strategy: shard n_routes (4096) across 8 cores (512 routes/core).
All bulk data is bf16 (tolerance 2e-2; measured end-to-end ~2e-3):
  - u_hat produced once per core via PE matmuls (block-diagonal x as
    stationary, host-pre-transposed W[(rb,i),(o,c)] as moving) and kept
    resident in SBUF as [p=(rb,b), (o,g,c)] bf16 (64 KB/partition).
    The (o,g,c) free order keeps c innermost (stride 1) so every big
    DVE tensor_tensor runs in 2x_1p mode.
  - s0 accumulated straight from the staged W tiles with a compact
    x/R stationary (independent of the PSUM->SBUF copies).
  - routing iterations: DVE mults (u*v_rep, u*e) in bf16; all
    cross-partition sums AND the o-reduction run on the PE via
    PSUM-accumulated delta/mean matmuls; softmax exp on the scalar
    engine; tiny f32 AllReduce ([16,544]: route-sum + softmax denom)
    per iteration.
"""

import numpy as np

B, R, I, C, O = 16, 4096, 16, 32, 16
NCORES = 8
RL = R // NCORES      # 512 routes per core
G = RL // 8           # 64 groups of 8 routes
CO = C * O            # 512
GPB = 8               # groups per DMA block
NBLK = G // GPB       # 8 blocks
SEG = CO + 128 + 16   # per-group stage: wt(512) + xb(128) + xs(16)
CHG = 8               # groups per DVE chunk
NCH = G // CHG        # 8 chunks
GPR = 16              # groups per a-phase PSUM region
NREG = G // GPR       # 4 regions

_cache = {}


def _patch_tile_drain():
    import concourse.tile as tile_mod
    from concourse.vector_clock import ScopedClock, VectorClock

    if getattr(tile_mod.TileContext, "_drain_patched", False):
        return

    def _split_drain_and_barrier(self, tick_clock, wait_clock):
        ticks = list(tick_clock.global_clock)
        for i in [j for j, t in enumerate(ticks) if t > 0]:
            vec = [ticks[j] if j == i else 0 for j in range(len(ticks))]
            d = self.nc.sync.drain()
            wait_clock.add_sem_waits(d.ins, ScopedClock({None: VectorClock(vec)}))
        self.nc.all_engine_barrier()
        popped = self.nc._tile_sem_poison_stack.pop()
        assert popped is self._sem_poison
        self.nc.clear_and_free_semaphores(list(self.sems.allocated().values()))
        self.nc.all_engine_barrier()

    tile_mod.TileContext._drain_and_barrier = _split_drain_and_barrier
    tile_mod.TileContext._drain_patched = True


def _split_waits(nc, limit=1):
    """This container's walrus rejects >1 sync-wait per instruction; move
    excess waits onto same-engine NoOps inserted just before the owner."""
    import concourse.mybir as mybir

    blocks = nc.main_func.blocks
    for bb in blocks:
        insts = bb.instructions  # live list view
        k = 0
        while k < len(insts):
            inst = insts[k]
            si = inst.sync_info
            if si is not None and si.on_wait and len(si.on_wait) > limit:
                w = list(si.on_wait)
                si.on_wait = w[:limit]
                excess = w[limit:]
                insert_at = k
                for cs in range(0, len(excess), limit):
                    chunk = excess[cs:cs + limit]
                    nop = nc.engines[inst.engine].nop()
                    ni = nop.ins
                    for bb2 in blocks:
                        l2 = bb2.instructions
                        hit = next(
                            (i for i in range(len(l2) - 1, -1, -1)
                             if l2[i].name == ni.name), None)
                        if hit is not None:
                            l2.pop(hit)
                            break
                    ni.sync_info = mybir.SyncInfo(on_wait=chunk, on_update=[])
                    insts.insert(insert_at, ni)
                    insert_at += 1
                    k += 1
            k += 1


def _build_nc():
    import concourse.bass as bass
    import concourse.mybir as mybir
    from concourse.tile import TileContext

    _patch_tile_drain()
    F32 = mybir.dt.float32
    BF16 = mybir.dt.bfloat16
    AF = mybir.ActivationFunctionType
    ALU = mybir.AluOpType
    CORES = list(range(NCORES))

    nc = bass.Bass(target_bir_lowering=False)
    wx_d = nc.declare_dram_parameter("wx", [NBLK, 128, GPB * SEG], BF16,
                                     isOutput=False)
    db_d = nc.declare_dram_parameter("delta_b", [128, 16], BF16, isOutput=False)
    ob_d = nc.declare_dram_parameter("ones_bd", [128, 128], BF16, isOutput=False)
    o16_d = nc.declare_dram_parameter("ones_16", [128, 16], F32, isOutput=False)
    dr_d = nc.declare_dram_parameter("delta_rep", [16, 128], BF16, isOutput=False)
    out_d = nc.declare_dram_parameter("out", [16, CO], F32, isOutput=True)
    cc_in = [nc.dram_tensor(f"cc_in{k}", [16, 544], F32) for k in range(3)]
    cc_out = [
        nc.dram_tensor(f"cc_out{k}", [16, 544], F32, addr_space="Shared")
        for k in range(3)
    ]

    with TileContext(nc) as tc:
        with (
            tc.tile_pool(name="big", bufs=1) as big,
            tc.tile_pool(name="stage", bufs=2) as stage,
            tc.tile_pool(name="small", bufs=1) as small,
            tc.tile_pool(name="psA", bufs=2, space="PSUM") as psA,
            tc.tile_pool(name="psS", bufs=1, space="PSUM") as psS,
            tc.tile_pool(name="psD", bufs=1, space="PSUM") as psD,
            tc.tile_pool(name="psAB", bufs=1, space="PSUM") as psAB,
        ):
            # constants
            db = small.tile([128, 16], BF16, tag="db")
            ob = small.tile([128, 128], BF16, tag="ob")
            o16 = small.tile([128, 16], F32, tag="o16")
            dr = small.tile([16, 128], BF16, tag="dr")
            nc.sync.dma_start(out=db[:, :], in_=db_d[:, :])
            nc.sync.dma_start(out=ob[:, :], in_=ob_d[:, :])
            nc.sync.dma_start(out=o16[:, :], in_=o16_d[:, :])
            nc.sync.dma_start(out=dr[:, :], in_=dr_d[:, :])

            u_sb = big.tile([128, O, G, C], BF16, tag="u")
            prod = big.tile([128, O, G, C], BF16, tag="prod")
            b_f = small.tile([128, G, C], F32, tag="b_f")
            e16 = small.tile([128, G, C], BF16, tag="e16")
            v_rep = small.tile([128, O, C], BF16, tag="v_rep")
            dnp = small.tile([128, C], F32, tag="dnp")
            nc.gpsimd.memset(b_f[:, :, :], 0.0)

            st = small.tile([16, 544], F32, tag="st")
            st2 = small.tile([16, 544], F32, tag="st2")
            nc.vector.memset(st[:, 512:], 0.0)

            # ---- production + s0 accumulation ----
            ps_s = psS.tile([16, CO], F32, tag="ps_s")
            for blk in range(NBLK):
                stg = stage.tile([128, GPB * SEG], BF16, tag="stg")
                eng = nc.sync if blk % 2 == 0 else nc.gpsimd
                eng.dma_start(out=stg[:, :], in_=wx_d[blk, :, :])
                for j in range(GPB):
                    g = blk * GPB + j
                    base = j * SEG
                    wt = stg[:, base:base + CO]
                    xb = stg[:, base + CO:base + CO + 128]
                    xs = stg[:, base + CO + 128:base + SEG]
                    pu = psA.tile([128, CO], F32, tag="pu")
                    nc.tensor.matmul(pu[:, :], xb, wt, start=True, stop=True)
                    # s0 += x/R @ wt  (reads the stage tile, not u_sb)
                    nc.tensor.matmul(ps_s[:, :], xs, wt,
                                     start=(g == 0), stop=(g == G - 1))
                    puv = pu[:, :].rearrange("p (o c) -> p o c", o=O)
                    if g % 2 == 0:
                        nc.vector.tensor_copy(u_sb[:, :, g, :], puv)
                    else:
                        nc.scalar.copy(u_sb[:, :, g, :], puv)

            def start_cc(it):
                nc.sync.dma_start(out=cc_in[it][:, :], in_=st[:, :])
                nc.gpsimd.collective_compute(
                    "AllReduce", ALU.add, replica_groups=[CORES],
                    ins=[cc_in[it][:, :]], outs=[cc_out[it][:, :]],
                )
                nc.sync.dma_start(out=st2[:, :], in_=cc_out[it][:, :])

            nc.vector.tensor_copy(st[:, :512], ps_s[:, :])
            start_cc(0)

            s_t = small.tile([16, CO], F32, tag="s_t")
            sq = small.tile([16, CO], F32, tag="sq")
            ab = small.tile([16, CO], F32, tag="ab")
            den = small.tile([16, CO], F32, tag="den")
            m1 = small.tile([16, CO], F32, tag="m1")
            v16 = small.tile([16, CO], BF16, tag="v16")
            v_f = small.tile([16, CO], F32, tag="v_f")
            rdn = small.tile([16, C], F32, tag="rdn")

            def squash(k):
                # v = s*|s| / (1+s^2)   (== reference squash elementwise)
                if k == 0:
                    s = st2[:, :512]
                else:
                    nc.vector.reciprocal(rdn[:, :], st2[:, 512:544])
                    nc.vector.tensor_tensor(
                        s_t[:, :].rearrange("p (o c) -> p o c", o=O),
                        st2[:, :512].rearrange("p (o c) -> p o c", o=O),
                        rdn[:, :].unsqueeze(1).broadcast_to([16, O, C]),
                        ALU.mult,
                    )
                    s = s_t[:, :]
                nc.scalar.activation(ab[:, :], s, AF.Abs)
                nc.vector.tensor_tensor(sq[:, :], s, s, ALU.mult)
                nc.vector.tensor_scalar_add(den[:, :], sq[:, :], 1.0)
                nc.vector.reciprocal(den[:, :], den[:, :])
                nc.vector.tensor_tensor(m1[:, :], s, ab[:, :], ALU.mult)
                out = v_f if k == 2 else v16
                nc.vector.tensor_tensor(out[:, :], m1[:, :], den[:, :], ALU.mult)

            for it in (1, 2):
                squash(it - 1)
                # v_rep[(rb,b), (o,c)] = v[b, (o,c)]
                ps_vr = psA.tile([128, CO], F32, tag="pu")
                nc.tensor.matmul(ps_vr[:, :], dr[:, :], v16[:, :],
                                 start=True, stop=True)
                nc.scalar.copy(
                    v_rep[:, :, :],
                    ps_vr[:, :].rearrange("p (o c) -> p o c", o=O),
                )

                # ---- a-phase: prod = u*v_rep (DVE); PE sums o + batch-mean
                ps_ab = psAB.tile([128, G, C], F32, tag="ps_ab")
                for ch in range(NCH):
                    gs = ch * CHG
                    nc.vector.tensor_tensor(
                        prod[:, :, gs:gs + CHG, :],
                        u_sb[:, :, gs:gs + CHG, :],
                        v_rep[:, :, :].unsqueeze(2).broadcast_to(
                            [128, O, CHG, C]),
                        ALU.mult,
                    )
                    if ch % 2 == 1:
                        g0 = gs - CHG
                        for o in range(O):
                            nc.tensor.matmul(
                                ps_ab[:, g0:g0 + GPR, :],
                                ob[:, :],
                                prod[:, o, g0:g0 + GPR, :],
                                start=(o == 0), stop=(o == O - 1),
                            )
                # b += batch-mean agreement
                nc.vector.tensor_tensor(
                    b_f[:, :, :], b_f[:, :, :], ps_ab[:, :, :], ALU.add,
                )
                nc.scalar.activation(
                    e16[:, :, :].rearrange("p g c -> p (g c)"),
                    b_f[:, :, :].rearrange("p g c -> p (g c)"),
                    AF.Exp,
                )
                # local softmax denominator (innermost-g strided view)
                nc.vector.tensor_reduce(
                    dnp[:, :],
                    e16[:, :, :].transpose([0, 2, 1]),
                    mybir.AxisListType.X, ALU.add,
                )
                ps_dn = psD.tile([16, C], F32, tag="ps_dn")
                nc.tensor.matmul(ps_dn[:, :], o16[:, :], dnp[:, :],
                                 start=True, stop=True)

                # ---- s-phase: prod = u*e (DVE); PE delta-sums routes
                for ch in range(NCH):
                    gs = ch * CHG
                    nc.vector.tensor_tensor(
                        prod[:, :, gs:gs + CHG, :],
                        u_sb[:, :, gs:gs + CHG, :],
                        e16[:, gs:gs + CHG, :].unsqueeze(1).broadcast_to(
                            [128, O, CHG, C]),
                        ALU.mult,
                    )
                    for j in range(CHG):
                        g = gs + j
                        nc.tensor.matmul(
                            ps_s[:, :], db[:, :], prod[:, :, g, :],
                            start=(g == 0), stop=(g == G - 1),
                        )
                nc.vector.tensor_copy(st[:, :512], ps_s[:, :])
                nc.scalar.copy(st[:, 512:544], ps_dn[:, :])
                start_cc(it)

            squash(2)
            nc.sync.dma_start(out=out_d[:, :], in_=v_f[:, :])

    _split_waits(nc)
    return nc


def _prep_inputs(x, W):
    import ml_dtypes

    BF = ml_dtypes.bfloat16
    x = np.ascontiguousarray(x, np.float32)
    W = np.ascontiguousarray(W, np.float32)
    # wt[core, g, (rb,i), (o,c)] = W[r=(core,g,rb), c, o, i]
    Wv = W.reshape(NCORES, G, 8, C, O, I)
    wt = np.ascontiguousarray(
        Wv.transpose(0, 1, 2, 5, 4, 3).reshape(NCORES, G, 128, CO)
    ).astype(BF)
    # xv[core, g, rb, i, b] = x[b, r, i]
    xv = np.ascontiguousarray(x.transpose(1, 2, 0)).reshape(NCORES, G, 8, I, B)
    xb = np.zeros((NCORES, G, 128, 128), np.float32)
    for rb in range(8):
        xb[:, :, rb * 16:(rb + 1) * 16, rb * 16:(rb + 1) * 16] = xv[:, :, rb]
    xb = xb.astype(BF)
    xs = (xv.reshape(NCORES, G, 128, 16) / np.float32(R)).astype(BF)
    wx = np.zeros((NCORES, NBLK, 128, GPB * SEG), BF)
    for j in range(GPB):
        base = j * SEG
        wx[:, :, :, base:base + CO] = wt.reshape(NCORES, NBLK, GPB, 128, CO)[:, :, j]
        wx[:, :, :, base + CO:base + CO + 128] = \
            xb.reshape(NCORES, NBLK, GPB, 128, 128)[:, :, j]
        wx[:, :, :, base + CO + 128:base + SEG] = \
            xs.reshape(NCORES, NBLK, GPB, 128, 16)[:, :, j]
    db = np.tile(np.eye(16, dtype=np.float32), (8, 1)).astype(BF)   # [128,16]
    ob = np.kron(np.eye(8, dtype=np.float32),
                 np.full((16, 16), 1.0 / B, np.float32)).astype(BF)  # [128,128]
    o16 = np.full((128, 16), 1.0 / 16.0, np.float32)
    dr = np.tile(np.eye(16, dtype=np.float32), (1, 8)).astype(BF)    # [16,128]
    in_maps = []
    for c in range(NCORES):
        in_maps.append({
            "wx": wx[c],
            "delta_b": db, "ones_bd": ob,
            "ones_16": o16, "delta_rep": dr,
        })
    return in_maps


def kernel(x, W):
    from concourse.bass_utils import run_bass_kernel_spmd

    if "nc" not in _cache:
        _cache["nc"] = _build_nc()
    in_maps = _prep_inputs(x, W)
    res = run_bass_kernel_spmd(_cache["nc"], in_maps, list(range(NCORES)))
    # out is [b, (o,c)] -> reference layout [b, c, o, 1]
    v = res.results[0]["out"].reshape(B, O, C).transpose(0, 2, 1)[..., None]
    return np.ascontiguousarray(v, np.float32)
